# revision 16
# baseline (speedup 1.0000x reference)
"""Masked attention kernel for Trainium2, data-parallel over batch on 8 NeuronCores.

Problem (per reference):
    query (128, 512) f32, key/value (1024, 128, 512) f32, mask (128, 1, 1024) i32
    energy = einsum('bh,tbh->bt'); attn = softmax(energy)
    masked = mask*attn / sum(mask*attn); context = einsum('bt,tbh->bh')
    returns (context (128,512), masked_attention (128,1024))

Key algebraic simplification: the unmasked softmax normalizer cancels:
    masked = m*exp(e-max) / sum(m*exp(e-max))

Per-core structure (B_loc=16, T=1024, H=512; t on partitions, 8 t-tiles):
    - q broadcast on-chip: 2 KB row DMAs + PE outer products (ones^T @ q_row)
    - K streamed as (128 t, 8 half-batch, 512 h) tiles: 16 KB contiguous
      bursts at 32 KB stride, 2 MB per DMA
    - energy on DVE: big tensor_tensor mult + 3D tensor_reduce over h
      -> E_all (128 t-part, tt*16+b columns)
    - PE transposes E to row layout (16 b-part, 1024 t); softmax row-wise:
      reduce_max(negate) -> ACT Exp(bias=-max) -> mask mult + sum -> reciprocal
    - attn rows scaled by 1/Z, DMA'd out; attn transposed back to columns
    - context via PE matmuls in float32r (lhsT = attn column, rhs = V tile),
      one (1,512) psum row per batch, copied and DMA'd per row
"""

import numpy as np

B, T, H = 128, 1024, 512
NCORES = 8
BL = B // NCORES  # 16 batches per core
NT = T // 128     # 8 t-tiles
HB = BL // 2      # half-batch chunk (K tiles)
QB = BL // 4      # quarter-batch chunk (product scratch)

_cache = {}


def _build_nc(debug=False):
    from contextlib import ExitStack

    import concourse.bacc as bacc
    import concourse.bass as bass
    import concourse.mybir as mybir
    import concourse.tile as tile
    from concourse import masks

    f32 = mybir.dt.float32
    f32r = mybir.dt.float32r
    i32 = mybir.dt.int32
    Alu = mybir.AluOpType
    Act = mybir.ActivationFunctionType

    nc = bacc.Bacc("TRN2", target_bir_lowering=False, debug=debug)

    q_d = nc.dram_tensor("query", [BL, H], f32, kind="ExternalInput")
    k_d = nc.dram_tensor("key", [T, BL, H], f32, kind="ExternalInput")
    v_d = nc.dram_tensor("value", [T, BL, H], f32r, kind="ExternalInput")
    m_d = nc.dram_tensor("mask", [BL, 1, T], i32, kind="ExternalInput")
    ctx_d = nc.dram_tensor("out_ctx", [BL, H], f32, kind="ExternalOutput")
    attn_d = nc.dram_tensor("out_attn", [BL, T], f32, kind="ExternalOutput")

    # K: (T, BL, H) -> (NT, 2, 128, HB, H): per (t-tile, half-batch) tiles.
    # Each k_r[tt, hf] = 128 partitions x 16 KB contiguous, 32 KB stride.
    k_r = k_d.ap().rearrange("(tt p) (hf b) h -> tt hf p b h", p=128, b=HB)
    # V: per (b, tt) 256 KB slices; descriptors stride 32 KB
    v_r = v_d.ap().rearrange("(tt p) b h -> b tt p h", p=128)

    with tile.TileContext(nc) as tc, ExitStack() as ctx:
        const = ctx.enter_context(tc.tile_pool(name="const", bufs=1))
        once = ctx.enter_context(tc.tile_pool(name="once", bufs=1))
        kpool = ctx.enter_context(tc.tile_pool(name="kpool", bufs=3))
        vpool = ctx.enter_context(tc.tile_pool(name="vpool", bufs=2))
        prodp = ctx.enter_context(tc.tile_pool(name="prodp", bufs=2))
        psum_q = ctx.enter_context(
            tc.tile_pool(name="psum_q", bufs=1, space=bass.MemorySpace.PSUM)
        )
        psum_e = ctx.enter_context(
            tc.tile_pool(name="psum_e", bufs=1, space=bass.MemorySpace.PSUM)
        )
        psum_w = ctx.enter_context(
            tc.tile_pool(name="psum_w", bufs=1, space=bass.MemorySpace.PSUM)
        )
        psum_c = ctx.enter_context(
            tc.tile_pool(name="psum_c", bufs=4, space=bass.MemorySpace.PSUM)
        )

        identity = const.tile([128, 128], f32)
        masks.make_identity(nc, identity[:])

        mask_i = once.tile([BL, T], i32)
        nc.sync.dma_start(mask_i[:], m_d.ap().rearrange("b o t -> b (o t)"))
        mask_f = const.tile([BL, T], f32)
        nc.vector.tensor_copy(mask_f[:], mask_i[:])

        # broadcast each query row to all 128 partitions: DMA each row to
        # partition 0, then PE outer-product (ones^T @ q_row)
        ones = const.tile([1, 128], f32)
        nc.vector.memset(ones[:], 1.0)
        qb = const.tile([128, BL, H], f32)
        for b in range(BL):
            q_row = prodp.tile([1, H], f32, tag="qrow")
            nc.sync.dma_start(q_row[:], q_d.ap()[b : b + 1, :])
            qb_ps = psum_q.tile([128, H], f32)
            nc.tensor.matmul(qb_ps[:], ones[:], q_row[:], start=True, stop=True)
            nc.scalar.copy(qb[:, b, :], qb_ps[:])

        # ---- energy: E_all[p, tt*BL+b] = sum_h K[tt*128+p, b, h] * q[b, h]
        E_all = const.tile([128, NT * BL], f32)
        for tt in range(NT):
            for hf in range(2):
                kt = kpool.tile([128, HB, H], f32, tag="kt")
                nc.sync.dma_start(kt[:], k_r[tt, hf])
                for q4 in range(HB // QB):
                    b0 = hf * HB + q4 * QB
                    pr = prodp.tile([128, QB, H], f32, tag="pr")
                    nc.vector.tensor_tensor(
                        out=pr[:],
                        in0=kt[:, q4 * QB : (q4 + 1) * QB, :],
                        in1=qb[:, b0 : b0 + QB, :],
                        op=Alu.mult,
                    )
                    nc.vector.tensor_reduce(
                        E_all[:, tt * BL + b0 : tt * BL + b0 + QB],
                        pr[:],
                        axis=mybir.AxisListType.X,
                        op=Alu.add,
                    )

        # ---- transpose energy to row layout: erow[b, t]
        erow = psum_e.tile([BL, T], f32)
        for tt in range(NT):
            nc.tensor.transpose(
                erow[:, tt * 128 : (tt + 1) * 128],
                E_all[:, tt * BL : (tt + 1) * BL],
                identity[:],
            )

        # ---- softmax (row-wise over free dim)
        negmax = const.tile([BL, 1], f32)
        nc.vector.tensor_reduce(
            negmax[:], erow[:], axis=mybir.AxisListType.X, op=Alu.max, negate=True
        )
        xrow = once.tile([BL, T], f32)
        nc.scalar.activation(xrow[:], erow[:], Act.Exp, bias=negmax[:], scale=1.0)
        wrow = const.tile([BL, T], f32)
        zsum = const.tile([BL, 1], f32)
        nc.vector.tensor_tensor(out=wrow[:], in0=xrow[:], in1=mask_f[:], op=Alu.mult)
        nc.vector.tensor_reduce(
            zsum[:], wrow[:], axis=mybir.AxisListType.X, op=Alu.add
        )
        rz = const.tile([BL, 1], f32)
        nc.vector.reciprocal(rz[:], zsum[:])
        attn = const.tile([BL, T], f32)
        nc.vector.tensor_scalar_mul(attn[:], wrow[:], rz[:])
        nc.sync.dma_start(attn_d.ap(), attn[:])

        # ---- transpose normalized attn to column layout:
        #      wcol[p, tt*BL+b] = attn[b, tt*128+p]  (already scaled by 1/Z)
        wcol_ps = psum_w.tile([128, NT * BL], f32)
        for tt in range(NT):
            nc.tensor.transpose(
                wcol_ps[:, tt * BL : (tt + 1) * BL],
                attn[:, tt * 128 : (tt + 1) * 128],
                identity[:BL, :BL],
            )
        wcol = const.tile([128, NT * BL], f32r)
        nc.scalar.copy(wcol[:], wcol_ps[:])

        # ---- context: ctx[b, h] = sum_t attn[b, t] * V[t, b, h] (float32r matmuls)
        for b in range(BL):
            vt = vpool.tile([128, NT, H], f32r, tag="vt")
            for tt in range(NT):
                nc.sync.dma_start(vt[:, tt, :], v_r[b, tt])
            cps = psum_c.tile([1, H], f32)
            for tt in range(NT):
                nc.tensor.matmul(
                    cps[:],
                    wcol[:, tt * BL + b : tt * BL + b + 1],
                    vt[:, tt, :],
                    start=(tt == 0),
                    stop=(tt == NT - 1),
                )
            crow = prodp.tile([1, H], f32, tag="crow")
            nc.scalar.copy(crow[:], cps[:])
            nc.sync.dma_start(ctx_d.ap()[b : b + 1, :], crow[:])

    nc.compile()
    return nc


def _get_nc():
    if "nc" not in _cache:
        _cache["nc"] = _build_nc(debug=False)
    return _cache["nc"]


def _shard_inputs(query, key, value, mask):
    in_maps = []
    for i in range(NCORES):
        s = slice(i * BL, (i + 1) * BL)
        in_maps.append(
            {
                "query": np.ascontiguousarray(query[s]),
                "key": np.ascontiguousarray(key[:, s]),
                "value": np.ascontiguousarray(value[:, s]),
                "mask": np.ascontiguousarray(mask[s]),
            }
        )
    return in_maps


def run_sharded(query, key, value, mask, trace=False, **kw):
    from concourse.bass_utils import run_bass_kernel_spmd

    nc = _get_nc()
    in_maps = _shard_inputs(query, key, value, mask)
    res = run_bass_kernel_spmd(
        nc, in_maps, core_ids=list(range(NCORES)), trace=trace, **kw
    )
    context = np.concatenate([res.results[i]["out_ctx"] for i in range(NCORES)], axis=0)
    attn = np.concatenate([res.results[i]["out_attn"] for i in range(NCORES)], axis=0)
    return (context, attn), res


def kernel(query, key, value, mask):
    query = np.asarray(query, dtype=np.float32)
    key = np.asarray(key, dtype=np.float32)
    value = np.asarray(value, dtype=np.float32)
    mask = np.asarray(mask, dtype=np.int32)
    (context, attn), _ = run_sharded(query, key, value, mask, trace=False)
    return (context, attn)


# revision 17
# speedup vs baseline: 1.3523x; 1.3523x over previous
"""Masked attention kernel for Trainium2, data-parallel over batch on 8 NeuronCores.

Problem (per reference):
    query (128, 512) f32, key/value (1024, 128, 512) f32, mask (128, 1, 1024) i32
    energy = einsum('bh,tbh->bt'); attn = softmax(energy)
    masked = mask*attn / sum(mask*attn); context = einsum('bt,tbh->bh')
    returns (context (128,512), masked_attention (128,1024))

Key algebraic simplification: the unmasked softmax normalizer cancels:
    masked = m*exp(e-max) / sum(m*exp(e-max))

Per-core structure (B_loc=16, T=1024, H=512; t on partitions, 8 t-tiles):
    - q broadcast on-chip: 2 KB row DMAs + PE outer products (ones^T @ q_row)
    - K and V streamed as (128 t, 8 half-batch, 512 h) tiles: 16 KB contiguous
      bursts at 32 KB stride, 2 MB per DMA
    - energy: fused DVE affine_mul_reduce (K*q_bcast, sum over h) one pass
      -> E_all (128 t-part, tt*16+b columns)
    - PE transposes E to row layout (16 b-part, 1024 t); softmax row-wise:
      reduce_max(negate) -> ACT Exp(bias=-max) -> mask mult + sum -> reciprocal
    - attn rows scaled by 1/Z, DMA'd out; attn transposed back to columns
    - context: all 128 PE matmuls (float32r) accumulate into ONE (16,512)
      psum tile; lhsT is column-masked (only column b nonzero) so each row
      accumulates exactly its own batch. One copy + one 32 KB output DMA.
"""

import numpy as np

B, T, H = 128, 1024, 512
NCORES = 8
BL = B // NCORES  # 16 batches per core
NT = T // 128     # 8 t-tiles
HB = BL // 2      # half-batch chunk (K/V tiles)

_cache = {}


def _build_nc(debug=False):
    from contextlib import ExitStack

    import concourse.bacc as bacc
    import concourse.bass as bass
    import concourse.mybir as mybir
    import concourse.tile as tile
    from concourse import masks

    f32 = mybir.dt.float32
    f32r = mybir.dt.float32r
    i32 = mybir.dt.int32
    Alu = mybir.AluOpType
    Act = mybir.ActivationFunctionType

    nc = bacc.Bacc("TRN2", target_bir_lowering=False, debug=debug)

    q_d = nc.dram_tensor("query", [BL, H], f32, kind="ExternalInput")
    k_d = nc.dram_tensor("key", [T, BL, H], f32, kind="ExternalInput")
    v_d = nc.dram_tensor("value", [T, BL, H], f32r, kind="ExternalInput")
    m_d = nc.dram_tensor("mask", [BL, 1, T], i32, kind="ExternalInput")
    ctx_d = nc.dram_tensor("out_ctx", [BL, H], f32, kind="ExternalOutput")
    attn_d = nc.dram_tensor("out_attn", [BL, T], f32, kind="ExternalOutput")

    # (T, BL, H) -> (NT, 2, 128, HB, H): per (t-tile, half-batch) tiles.
    # Each [tt, hf] slice = 128 partitions x 16 KB contiguous, 32 KB stride.
    k_r = k_d.ap().rearrange("(tt p) (hf b) h -> tt hf p b h", p=128, b=HB)
    v_r = v_d.ap().rearrange("(tt p) (hf b) h -> tt hf p b h", p=128, b=HB)

    with tile.TileContext(nc) as tc, ExitStack() as ctx:
        const = ctx.enter_context(tc.tile_pool(name="const", bufs=1))
        once = ctx.enter_context(tc.tile_pool(name="once", bufs=1))
        kpool = ctx.enter_context(tc.tile_pool(name="kpool", bufs=3))
        vpool = ctx.enter_context(tc.tile_pool(name="vpool", bufs=3))
        prodp = ctx.enter_context(tc.tile_pool(name="prodp", bufs=2))
        psum_q = ctx.enter_context(
            tc.tile_pool(name="psum_q", bufs=2, space=bass.MemorySpace.PSUM)
        )
        psum_e = ctx.enter_context(
            tc.tile_pool(name="psum_e", bufs=1, space=bass.MemorySpace.PSUM)
        )
        psum_w = ctx.enter_context(
            tc.tile_pool(name="psum_w", bufs=1, space=bass.MemorySpace.PSUM)
        )
        psum_c = ctx.enter_context(
            tc.tile_pool(name="psum_c", bufs=1, space=bass.MemorySpace.PSUM)
        )

        identity = const.tile([128, 128], f32)
        masks.make_identity(nc, identity[:])

        mask_i = once.tile([BL, T], i32)
        nc.sync.dma_start(mask_i[:], m_d.ap().rearrange("b o t -> b (o t)"))
        mask_f = const.tile([BL, T], f32)
        nc.vector.tensor_copy(mask_f[:], mask_i[:])

        # one-hot column masks for the context accumulation:
        # colmask[:, b*BL + j] = 1.0 iff j == b
        colmask = const.tile([128, BL * BL], f32)
        nc.gpsimd.memset(colmask[:], 0.0)
        for b in range(BL):
            nc.vector.memset(colmask[:, b * BL + b : b * BL + b + 1], 1.0)

        # broadcast each query row to all 128 partitions: DMA each row to
        # partition 0, then PE outer-product (ones^T @ q_row)
        ones = const.tile([1, 128], f32)
        nc.vector.memset(ones[:], 1.0)
        qb = const.tile([128, BL, H], f32)
        for b in range(BL):
            q_row = prodp.tile([1, H], f32, tag="qrow")
            nc.sync.dma_start(q_row[:], q_d.ap()[b : b + 1, :])
            qb_ps = psum_q.tile([128, H], f32)
            nc.tensor.matmul(qb_ps[:], ones[:], q_row[:], start=True, stop=True)
            nc.scalar.copy(qb[:, b, :], qb_ps[:])

        # ---- energy: E_all[p, tt*BL+b] = sum_h K[tt*128+p, b, h] * q[b, h]
        E_all = const.tile([128, NT * BL], f32)
        for tt in range(NT):
            for hf in range(2):
                kt = kpool.tile([128, HB, H], f32, tag="kt")
                nc.sync.dma_start(kt[:], k_r[tt, hf])
                for j in range(HB):
                    b = hf * HB + j
                    pr = prodp.tile([128, H], f32, tag="pr")
                    nc.vector.affine_mul_reduce(
                        out=pr[:],
                        accum_out=E_all[:, tt * BL + b : tt * BL + b + 1],
                        in0=kt[:, j, :],
                        in1=qb[:, b, :],
                        scale=1.0,
                        bias=0.0,
                    )

        # ---- transpose energy to row layout: erow[b, t]
        erow = psum_e.tile([BL, T], f32)
        for tt in range(NT):
            nc.tensor.transpose(
                erow[:, tt * 128 : (tt + 1) * 128],
                E_all[:, tt * BL : (tt + 1) * BL],
                identity[:],
            )

        # ---- softmax (row-wise over free dim)
        negmax = const.tile([BL, 1], f32)
        nc.vector.tensor_reduce(
            negmax[:], erow[:], axis=mybir.AxisListType.X, op=Alu.max, negate=True
        )
        xrow = once.tile([BL, T], f32)
        nc.scalar.activation(xrow[:], erow[:], Act.Exp, bias=negmax[:], scale=1.0)
        wrow = const.tile([BL, T], f32)
        zsum = const.tile([BL, 1], f32)
        nc.vector.tensor_tensor(out=wrow[:], in0=xrow[:], in1=mask_f[:], op=Alu.mult)
        nc.vector.tensor_reduce(
            zsum[:], wrow[:], axis=mybir.AxisListType.X, op=Alu.add
        )
        rz = const.tile([BL, 1], f32)
        nc.vector.reciprocal(rz[:], zsum[:])
        attn = const.tile([BL, T], f32)
        nc.vector.tensor_scalar_mul(attn[:], wrow[:], rz[:])
        nc.sync.dma_start(attn_d.ap(), attn[:])

        # ---- transpose normalized attn to column layout:
        #      wcol[p, tt*BL+b] = attn[b, tt*128+p]  (already scaled by 1/Z)
        wcol_ps = psum_w.tile([128, NT * BL], f32)
        for tt in range(NT):
            nc.tensor.transpose(
                wcol_ps[:, tt * BL : (tt + 1) * BL],
                attn[:, tt * 128 : (tt + 1) * 128],
                identity[:BL, :BL],
            )
        wcol = const.tile([128, NT * BL], f32)
        nc.scalar.copy(wcol[:], wcol_ps[:])

        # ---- context: ctx[b, h] = sum_t attn[b, t] * V[t, b, h]
        # All 128 float32r matmuls accumulate into one (16,512) psum tile.
        # lhsT for (tt, b) is wcol's tt block masked to column b only, so
        # psum row b accumulates exactly batch b's contributions.
        cps = psum_c.tile([BL, H], f32)
        nmm = NT * BL
        i = 0
        for tt in range(NT):
            for hf in range(2):
                vt = vpool.tile([128, HB, H], f32r, tag="vt")
                nc.sync.dma_start(vt[:], v_r[tt, hf])
                for j in range(HB):
                    b = hf * HB + j
                    lhsT = prodp.tile([128, BL], f32r, tag="lhsT")
                    nc.vector.tensor_tensor(
                        out=lhsT[:],
                        in0=wcol[:, tt * BL : (tt + 1) * BL],
                        in1=colmask[:, b * BL : (b + 1) * BL],
                        op=Alu.mult,
                    )
                    nc.tensor.matmul(
                        cps[:],
                        lhsT[:],
                        vt[:, j, :],
                        start=(i == 0),
                        stop=(i == nmm - 1),
                    )
                    i += 1
        ctx_sb = const.tile([BL, H], f32)
        nc.scalar.copy(ctx_sb[:], cps[:])
        nc.sync.dma_start(ctx_d.ap(), ctx_sb[:])

    nc.compile()
    return nc


def _get_nc():
    if "nc" not in _cache:
        _cache["nc"] = _build_nc(debug=False)
    return _cache["nc"]


def _shard_inputs(query, key, value, mask):
    in_maps = []
    for i in range(NCORES):
        s = slice(i * BL, (i + 1) * BL)
        in_maps.append(
            {
                "query": np.ascontiguousarray(query[s]),
                "key": np.ascontiguousarray(key[:, s]),
                "value": np.ascontiguousarray(value[:, s]),
                "mask": np.ascontiguousarray(mask[s]),
            }
        )
    return in_maps


def run_sharded(query, key, value, mask, trace=False, **kw):
    from concourse.bass_utils import run_bass_kernel_spmd

    nc = _get_nc()
    in_maps = _shard_inputs(query, key, value, mask)
    res = run_bass_kernel_spmd(
        nc, in_maps, core_ids=list(range(NCORES)), trace=trace, **kw
    )
    context = np.concatenate([res.results[i]["out_ctx"] for i in range(NCORES)], axis=0)
    attn = np.concatenate([res.results[i]["out_attn"] for i in range(NCORES)], axis=0)
    return (context, attn), res


def kernel(query, key, value, mask):
    query = np.asarray(query, dtype=np.float32)
    key = np.asarray(key, dtype=np.float32)
    value = np.asarray(value, dtype=np.float32)
    mask = np.asarray(mask, dtype=np.int32)
    (context, attn), _ = run_sharded(query, key, value, mask, trace=False)
    return (context, attn)


# revision 23
# speedup vs baseline: 1.4492x; 1.0716x over previous
"""Masked attention kernel for Trainium2, data-parallel over batch on 8 NeuronCores.

Problem (per reference):
    query (128, 512) f32, key/value (1024, 128, 512) f32, mask (128, 1, 1024) i32
    energy = einsum('bh,tbh->bt'); attn = softmax(energy)
    masked = mask*attn / sum(mask*attn); context = einsum('bt,tbh->bh')
    returns (context (128,512), masked_attention (128,1024))

Key algebraic simplification: the unmasked softmax normalizer cancels:
    masked = m*exp(e-max) / sum(m*exp(e-max))

Per-core structure (B_loc=16, T=1024, H=512; t on partitions, 8 t-tiles):
    - q broadcast on-chip: 2 KB row DMAs + PE outer products (ones^T @ q_row)
    - K and V streamed as (128 t, 8 half-batch, 512 h) tiles: 16 KB contiguous
      bursts at 32 KB stride, 2 MB per DMA
    - energy: fused DVE affine_mul_reduce (K*q_bcast, sum over h) one pass
      -> E_all (128 t-part, tt*16+b columns)
    - PE transposes E to row layout (16 b-part, 1024 t); softmax row-wise:
      reduce_max(negate) -> ACT Exp(bias=-max) -> mask mult + sum -> reciprocal
    - attn rows scaled by 1/Z, DMA'd out; attn transposed back to columns
    - context: all 128 PE matmuls (float32r) accumulate into ONE (16,512)
      psum tile; lhsT is column-masked (only column b nonzero) so each row
      accumulates exactly its own batch. One copy + one 32 KB output DMA.
"""

import numpy as np

B, T, H = 128, 1024, 512
NCORES = 8
BL = B // NCORES  # 16 batches per core
NT = T // 128     # 8 t-tiles
HB = BL // 2      # half-batch chunk (K/V tiles)

_cache = {}


def _build_nc(debug=False):
    from contextlib import ExitStack

    import concourse.bacc as bacc
    import concourse.bass as bass
    import concourse.mybir as mybir
    import concourse.tile as tile
    from concourse import masks

    f32 = mybir.dt.float32
    f32r = mybir.dt.float32r
    i32 = mybir.dt.int32
    Alu = mybir.AluOpType
    Act = mybir.ActivationFunctionType

    nc = bacc.Bacc("TRN2", target_bir_lowering=False, debug=debug)

    q_d = nc.dram_tensor("query", [BL, H], f32, kind="ExternalInput")
    k_d = nc.dram_tensor("key", [T, BL, H], f32, kind="ExternalInput")
    v_d = nc.dram_tensor("value", [T, BL, H], f32r, kind="ExternalInput")
    m_d = nc.dram_tensor("mask", [BL, 1, T], i32, kind="ExternalInput")
    ctx_d = nc.dram_tensor("out_ctx", [BL, H], f32, kind="ExternalOutput")
    attn_d = nc.dram_tensor("out_attn", [BL, T], f32, kind="ExternalOutput")

    # (T, BL, H) -> (NT, 2, 128, HB, H): per (t-tile, half-batch) tiles.
    # Each [tt, hf] slice = 128 partitions x 16 KB contiguous, 32 KB stride.
    k_r = k_d.ap().rearrange("(tt p) (hf b) h -> tt hf p b h", p=128, b=HB)
    v_r = v_d.ap().rearrange("(tt p) (hf b) h -> tt hf p b h", p=128, b=HB)

    with tile.TileContext(nc) as tc, ExitStack() as ctx:
        const = ctx.enter_context(tc.tile_pool(name="const", bufs=1))
        once = ctx.enter_context(tc.tile_pool(name="once", bufs=1))
        kpool = ctx.enter_context(tc.tile_pool(name="kpool", bufs=3))
        vpool = ctx.enter_context(tc.tile_pool(name="vpool", bufs=3))
        prodp = ctx.enter_context(tc.tile_pool(name="prodp", bufs=2))
        psum_q = ctx.enter_context(
            tc.tile_pool(name="psum_q", bufs=2, space=bass.MemorySpace.PSUM)
        )
        psum_e = ctx.enter_context(
            tc.tile_pool(name="psum_e", bufs=1, space=bass.MemorySpace.PSUM)
        )
        psum_w = ctx.enter_context(
            tc.tile_pool(name="psum_w", bufs=1, space=bass.MemorySpace.PSUM)
        )
        psum_c = ctx.enter_context(
            tc.tile_pool(name="psum_c", bufs=1, space=bass.MemorySpace.PSUM)
        )

        identity = const.tile([128, 128], f32)
        masks.make_identity(nc, identity[:])

        mask_i = once.tile([BL, T], i32)
        nc.sync.dma_start(mask_i[:], m_d.ap().rearrange("b o t -> b (o t)"))
        mask_f = const.tile([BL, T], f32)
        nc.vector.tensor_copy(mask_f[:], mask_i[:])

        # one-hot column masks for the context accumulation:
        # colmask[:, b*BL + j] = 1.0 iff j == b
        colmask = const.tile([128, BL * BL], f32)
        nc.gpsimd.memset(colmask[:], 0.0)
        for b in range(BL):
            nc.vector.memset(colmask[:, b * BL + b : b * BL + b + 1], 1.0)

        # broadcast each query row to all 128 partitions: two 16 KB DMAs land
        # the rows on partition 0 of two staging tiles, then one PE
        # outer-product (ones^T @ q_row) per batch
        ones = const.tile([1, 128], f32)
        nc.vector.memset(ones[:], 1.0)
        qa = []
        for hf in range(2):
            qa_t = kpool.tile([1, HB, H], f32, tag="kt")
            nc.sync.dma_start(
                qa_t[:], q_d.ap()[hf * HB : (hf + 1) * HB, :].unsqueeze(0)
            )
            qa.append(qa_t)
        qb = const.tile([128, BL, H], f32)
        for b in range(BL):
            src = qa[b // HB][0:1, b % HB, :]
            qb_ps = psum_q.tile([128, H], f32)
            nc.tensor.matmul(qb_ps[:], ones[:], src, start=True, stop=True)
            nc.scalar.copy(qb[:, b, :], qb_ps[:])

        # ---- energy: E_all[p, tt*BL+b] = sum_h K[tt*128+p, b, h] * q[b, h]
        # hf-outer so the first tiles only need half of qb built
        E_all = const.tile([128, NT * BL], f32)
        for hf in range(2):
            for tt in range(NT):
                kt = kpool.tile([128, HB, H], f32, tag="kt")
                nc.sync.dma_start(kt[:], k_r[tt, hf])
                for j in range(HB):
                    b = hf * HB + j
                    pr = prodp.tile([128, H], f32, tag="pr")
                    nc.vector.affine_mul_reduce(
                        out=pr[:],
                        accum_out=E_all[:, tt * BL + b : tt * BL + b + 1],
                        in0=kt[:, j, :],
                        in1=qb[:, b, :],
                        scale=1.0,
                        bias=0.0,
                    )

        # ---- transpose energy to row layout: erow[b, t]
        erow = psum_e.tile([BL, T], f32)
        for tt in range(NT):
            nc.tensor.transpose(
                erow[:, tt * 128 : (tt + 1) * 128],
                E_all[:, tt * BL : (tt + 1) * BL],
                identity[:],
            )

        # ---- softmax (row-wise over free dim)
        negmax = const.tile([BL, 1], f32)
        nc.vector.tensor_reduce(
            negmax[:], erow[:], axis=mybir.AxisListType.X, op=Alu.max, negate=True
        )
        xrow = once.tile([BL, T], f32)
        nc.scalar.activation(xrow[:], erow[:], Act.Exp, bias=negmax[:], scale=1.0)
        wrow = const.tile([BL, T], f32)
        zsum = const.tile([BL, 1], f32)
        nc.vector.tensor_tensor(out=wrow[:], in0=xrow[:], in1=mask_f[:], op=Alu.mult)
        nc.vector.tensor_reduce(
            zsum[:], wrow[:], axis=mybir.AxisListType.X, op=Alu.add
        )
        rz = const.tile([BL, 1], f32)
        nc.vector.reciprocal(rz[:], zsum[:])
        attn = const.tile([BL, T], f32)
        nc.vector.tensor_scalar_mul(attn[:], wrow[:], rz[:])
        # output DMAs go on the ACT queue: the sync queue is FIFO per engine,
        # and a compute-dependent DMA there would head-of-line block V loads
        nc.scalar.dma_start(attn_d.ap(), attn[:])

        # ---- transpose normalized attn to column layout:
        #      wcol[p, tt*BL+b] = attn[b, tt*128+p]  (already scaled by 1/Z)
        wcol_ps = psum_w.tile([128, NT * BL], f32)
        for tt in range(NT):
            nc.tensor.transpose(
                wcol_ps[:, tt * BL : (tt + 1) * BL],
                attn[:, tt * 128 : (tt + 1) * 128],
                identity[:BL, :BL],
            )
        wcol = const.tile([128, NT * BL], f32)
        nc.scalar.copy(wcol[:], wcol_ps[:])

        # ---- context: ctx[b, h] = sum_t attn[b, t] * V[t, b, h]
        # All 128 float32r matmuls accumulate into one (16,512) psum tile.
        # lhsT for (tt, b) is wcol's tt block masked to column b only, so
        # psum row b accumulates exactly batch b's contributions.
        cps = psum_c.tile([BL, H], f32)
        nmm = NT * BL
        i = 0
        for tt in range(NT):
            for hf in range(2):
                vt = vpool.tile([128, HB, H], f32r, tag="vt")
                nc.sync.dma_start(vt[:], v_r[tt, hf])
                for j in range(HB):
                    b = hf * HB + j
                    lhsT = prodp.tile([128, BL], f32r, tag="lhsT")
                    nc.vector.tensor_tensor(
                        out=lhsT[:],
                        in0=wcol[:, tt * BL : (tt + 1) * BL],
                        in1=colmask[:, b * BL : (b + 1) * BL],
                        op=Alu.mult,
                    )
                    nc.tensor.matmul(
                        cps[:],
                        lhsT[:],
                        vt[:, j, :],
                        start=(i == 0),
                        stop=(i == nmm - 1),
                    )
                    i += 1
        ctx_sb = const.tile([BL, H], f32)
        nc.scalar.copy(ctx_sb[:], cps[:])
        nc.scalar.dma_start(ctx_d.ap(), ctx_sb[:])

    nc.compile()
    return nc


def _get_nc():
    if "nc" not in _cache:
        _cache["nc"] = _build_nc(debug=False)
    return _cache["nc"]


def _shard_inputs(query, key, value, mask):
    in_maps = []
    for i in range(NCORES):
        s = slice(i * BL, (i + 1) * BL)
        in_maps.append(
            {
                "query": np.ascontiguousarray(query[s]),
                "key": np.ascontiguousarray(key[:, s]),
                "value": np.ascontiguousarray(value[:, s]),
                "mask": np.ascontiguousarray(mask[s]),
            }
        )
    return in_maps


def run_sharded(query, key, value, mask, trace=False, **kw):
    from concourse.bass_utils import run_bass_kernel_spmd

    nc = _get_nc()
    in_maps = _shard_inputs(query, key, value, mask)
    res = run_bass_kernel_spmd(
        nc, in_maps, core_ids=list(range(NCORES)), trace=trace, **kw
    )
    context = np.concatenate([res.results[i]["out_ctx"] for i in range(NCORES)], axis=0)
    attn = np.concatenate([res.results[i]["out_attn"] for i in range(NCORES)], axis=0)
    return (context, attn), res


def kernel(query, key, value, mask):
    query = np.asarray(query, dtype=np.float32)
    key = np.asarray(key, dtype=np.float32)
    value = np.asarray(value, dtype=np.float32)
    mask = np.asarray(mask, dtype=np.int32)
    (context, attn), _ = run_sharded(query, key, value, mask, trace=False)
    return (context, attn)
